# revision 3
# baseline (speedup 1.0000x reference)
"""BinaryLinear TRN2 kernel, v5 — pure-matmul PE, HWDGE-only DMA.

Computes out = inputs @ (sign(W) * scale).T + bias where
  sign(w) = +1 for w >= 0 else -1
  scale[o] = max(mean_i |W[o, i]|, 1e-6)

Shapes (hardcoded): inputs [8192, 4096] f32, weight [4096, 4096] f32,
bias [4096] f32 -> out [8192, 4096] f32. Data-parallel over tokens:
each of 8 cores takes [1024, 4096] of x plus full W/bias.

v5 vs v4 (743 us): Tile serializes DMA-transposes against every SWDGE
(gpsimd) DMA — the v4 trace shows xbars and gpsimd loads ping-ponging,
never overlapping, capping total DMA throughput below the matmul floor.
HWDGE plain DMAs DO run concurrently with transposes (v3 trace evidence),
so v5 uses NO gpsimd DMA at all:
  - scalar ring: all f32 loads (W halves + X halves), ACT Sign / Copy
    casts, scale round-trip + broadcast loads.
  - sync ring: all xbar transposes (half-tile [128,2048] -> [128,16,128])
    + output stores.
  - DVE: abs-row-sum reduces, psum evictions (x scale_bc, + bias_bc).
  - PE: ONLY the 2048 N=512 matmuls (437 us floor).
Scale is applied at EVICTION (v1-style DRAM round-trip broadcast), so the
W critical chain is just load -> sign -> xbar and the sign tiles stay
exactly +-1 in bf16 (no scale-rounding error).
"""

import os
import sys

import numpy as np

sys.path.insert(0, "/opt/trn_rl_repo")

import concourse.bass as bass
import concourse.mybir as mybir
from concourse import bacc
import concourse.tile as tile


def _ensure_ntff_hook():
    """The agent image's `antenv` lacks `axon_hooks`, which
    run_bass_kernel_spmd imports when trace=True (for HW exec timing).
    Provide the module and install the standard ctypes-based hook."""
    import types

    try:
        import antenv.axon_hooks  # noqa: F401
        return
    except ImportError:
        pass
    try:
        import antenv
    except ImportError:
        return
    mod = types.ModuleType("antenv.axon_hooks")
    state = {"hook": None}
    mod.set_axon_ntff_profile_hook = lambda h: state.update(hook=h)
    mod.get_axon_ntff_profile_hook = lambda: state["hook"]
    sys.modules["antenv.axon_hooks"] = mod
    antenv.axon_hooks = mod
    try:
        from trn_agent_boot.trn_boot import _ntff_profile_via_ctypes

        hook = _ntff_profile_via_ctypes("/opt/axon/libaxon_pjrt.so")
        if hook is not None:
            mod.set_axon_ntff_profile_hook(hook)
    except Exception:
        pass


_ensure_ntff_hook()

F32 = mybir.dt.float32
BF16 = mybir.dt.bfloat16

TOKENS = 8192
IN_FEATURES = 4096
OUT_FEATURES = 4096
N_CORES = 8


def build_nc(t_core, in_f, out_f, och=512):
    """Build the per-core Bass module. All cores run the identical program."""
    P = 128
    H = in_f // 2             # staging half-width (k)
    HK = H // P               # k-tiles per half
    KT = in_f // P            # contraction k-tiles
    TT = t_core // P          # token tiles
    OC = out_f // och         # output column chunks
    OT = och // P             # o-tiles (128 W rows) per chunk

    nc = bacc.Bacc()
    x_dram = nc.dram_tensor("x", [t_core, in_f], F32, kind="ExternalInput")
    w_dram = nc.dram_tensor("w", [out_f, in_f], F32, kind="ExternalInput")
    b_dram = nc.dram_tensor("b", [out_f], F32, kind="ExternalInput")
    out_dram = nc.dram_tensor("out", [t_core, out_f], F32, kind="ExternalOutput")

    with tile.TileContext(nc) as tc:
        with (
            tc.tile_pool(name="const", bufs=1) as const,
            tc.tile_pool(name="xt_pool", bufs=1) as xtp,
            tc.tile_pool(name="f32stage", bufs=5) as f32stage,
            tc.tile_pool(name="b16stage", bufs=4) as b16stage,
            tc.tile_pool(name="st", bufs=2) as stp,
            tc.tile_pool(name="small", bufs=4) as small,
            tc.tile_pool(name="bcast", bufs=4) as bcast,
            tc.tile_pool(name="outsb", bufs=3) as outsb,
            tc.tile_pool(name="psum_mm", bufs=6, space="PSUM") as psum_mm,
            tc.tile_pool(name="dram", bufs=1, space="DRAM") as dram_pool,
        ):
            # tiny positive bias so Sign(0 + tiny) = +1, matching the
            # reference's where(w >= 0, 1, -1)
            signbias = const.tile([P, 1], F32)
            nc.vector.memset(signbias[:], 1e-30)

            # per-row scale scratch in DRAM, written column-major per o-tile
            # ([o] viewed as [p, g]: column g holds rows g*128..g*128+127)
            scale_dram = dram_pool.tile([out_f], F32)
            scale_pm = scale_dram[:].rearrange("(g p) -> p g", p=P)

            # resident X^T, bf16: xt[kp, kt, t] = x[t, kt*128 + kp]
            xt = xtp.tile([P, KT, t_core], BF16)

            def build_x_half(t, h):
                xf = f32stage.tile([P, H], F32, tag="stage")
                nc.scalar.dma_start(
                    xf[:], x_dram[t * P:(t + 1) * P, h * H:(h + 1) * H]
                )
                xb = b16stage.tile([P, H], BF16, tag="b16")
                nc.scalar.activation(
                    xb[:], xf[:], mybir.ActivationFunctionType.Copy
                )
                nc.sync.dma_start(
                    xt[:, h * HK:(h + 1) * HK, t * P:(t + 1) * P],
                    xb[:], transpose=True,
                )

            def build_w_otile(st_c, ot, scale_cols):
                """One 128-row W tile: f32 half loads + Sign (scalar ring),
                abs-row-sum (DVE, off critical path), half xbars (sync)."""
                j = ot % OT
                red = small.tile([P, 2], F32, tag="red")
                for h in range(2):
                    wf = f32stage.tile([P, H], F32, tag="stage")
                    nc.scalar.dma_start(
                        wf[:], w_dram[ot * P:(ot + 1) * P, h * H:(h + 1) * H]
                    )
                    sg = b16stage.tile([P, H], BF16, tag="b16")
                    nc.scalar.activation(
                        sg[:], wf[:],
                        mybir.ActivationFunctionType.Sign, bias=signbias[:],
                    )
                    nc.vector.tensor_reduce(
                        red[:, h:h + 1], wf[:],
                        axis=mybir.AxisListType.X, op=mybir.AluOpType.add,
                        apply_absolute_value=True,
                    )
                    nc.sync.dma_start(
                        st_c[:, h * HK:(h + 1) * HK, j * P:(j + 1) * P],
                        sg[:], transpose=True,
                    )
                redt = small.tile([P, 1], F32, tag="redt")
                nc.vector.tensor_reduce(
                    redt[:], red[:],
                    axis=mybir.AxisListType.X, op=mybir.AluOpType.add,
                )
                nc.vector.tensor_scalar(
                    scale_cols[:, j:j + 1], redt[:], 1.0 / in_f, 1e-6,
                    op0=mybir.AluOpType.mult, op1=mybir.AluOpType.max,
                )

            def finish_chunk(oc, scale_cols):
                """Round-trip the chunk's scale to DRAM and broadcast-load it
                plus the bias slice as [128, och] partition-broadcast rows."""
                nc.scalar.dma_start(
                    scale_pm[:, oc * OT:(oc + 1) * OT], scale_cols[:]
                )
                sc_bc = bcast.tile([P, och], F32, tag="scbc")
                sslice = scale_dram[oc * och:(oc + 1) * och]
                nc.scalar.dma_start(
                    sc_bc[:],
                    bass.AP(tensor=sslice.tensor, offset=sslice.offset,
                            ap=[[0, P]] + list(sslice.ap)),
                )
                bias_c = bcast.tile([P, och], F32, tag="biasbc")
                bslice = b_dram[oc * och:(oc + 1) * och]
                nc.scalar.dma_start(
                    bias_c[:],
                    bass.AP(tensor=bslice.tensor, offset=bslice.offset,
                            ap=[[0, P]] + list(bslice.ap)),
                )
                return sc_bc, bias_c

            def mm_block(oc, t, st_c, sc_bc, bias_c):
                pm = psum_mm.tile([P, och], F32, tag="mmps")
                for kt in range(KT):
                    nc.tensor.matmul(
                        pm[:],
                        xt[:, kt, t * P:(t + 1) * P],
                        st_c[:, kt, :],
                        start=(kt == 0), stop=(kt == KT - 1),
                    )
                ob = outsb.tile([P, och], F32, tag="ob")
                nc.vector.tensor_mul(out=ob[:], in0=pm[:], in1=sc_bc[:])
                nc.vector.tensor_add(out=ob[:], in0=ob[:], in1=bias_c[:])
                nc.sync.dma_start(
                    out_dram[t * P:(t + 1) * P, oc * och:(oc + 1) * och],
                    ob[:],
                )

            def build_st_chunk_tiles(oc, st_c, scale_cols, tiles):
                for j in tiles:
                    build_w_otile(st_c, oc * OT + j, scale_cols)

            # chunk 0 upfront, X woven in for just-in-time arrival
            st0 = stp.tile([P, KT, och], BF16, tag="st")
            sc0_cols = small.tile([P, OT], F32, tag="scale_cols")
            build_st_chunk_tiles(0, st0, sc0_cols, range(OT))
            ctx0 = finish_chunk(0, sc0_cols)
            for t in range(TT):
                build_x_half(t, 0)
                build_x_half(t, 1)
            chunks = {0: (st0, *ctx0)}

            for oc in range(OC):
                if oc + 1 < OC:
                    st_n = stp.tile([P, KT, och], BF16, tag="st")
                    scn_cols = small.tile([P, OT], F32, tag="scale_cols")
                for t in range(TT):
                    mm_block(oc, t, *chunks[oc])
                    if oc + 1 < OC:
                        if 1 <= t <= OT:
                            build_w_otile(st_n, (oc + 1) * OT + (t - 1),
                                          scn_cols)
                        elif t == OT + 1:
                            chunks[oc + 1] = (st_n,
                                              *finish_chunk(oc + 1, scn_cols))
                del chunks[oc]

    nc.finalize()
    return nc


_CACHE = {}


def kernel(inputs, weight, bias):
    from concourse.bass_utils import run_bass_kernel_spmd

    x = np.ascontiguousarray(np.asarray(inputs, dtype=np.float32))
    w = np.ascontiguousarray(np.asarray(weight, dtype=np.float32))
    b = np.ascontiguousarray(np.asarray(bias, dtype=np.float32))
    assert x.shape == (TOKENS, IN_FEATURES)
    assert w.shape == (OUT_FEATURES, IN_FEATURES)
    assert b.shape == (OUT_FEATURES,)

    if "nc" not in _CACHE:
        _CACHE["nc"] = build_nc(TOKENS // N_CORES, IN_FEATURES, OUT_FEATURES)
    nc = _CACHE["nc"]

    shards = np.split(x, N_CORES, axis=0)
    in_maps = [{"x": shards[c], "w": w, "b": b} for c in range(N_CORES)]
    trace = bool(os.environ.get("BASS_TRACE"))
    res = run_bass_kernel_spmd(nc, in_maps, list(range(N_CORES)), trace=trace)
    if trace:
        _CACHE["last_result"] = res
        if res.exec_time_ns is not None:
            print(f"HW exec time: {res.exec_time_ns} ns")

    return np.concatenate([res.results[c]["out"] for c in range(N_CORES)], axis=0)
